# revision 3
# baseline (speedup 1.0000x reference)
"""ChebConv-with-spatial-attention Trainium2 kernel (8 NeuronCores, SPMD data-parallel).

Math (per batch b):
    M_k = cheb[k] * att[b]              (elementwise, [N,N])
    R_k = M_k @ xmat[b]                 (xmat[b][j, t*F+f] = x[b,t,j,f], [N, T*F])
    out[b,t,i,o] = relu( sum_k sum_f R_k[i, t*F+f] * Theta[k,f,o] )

Device mapping (per core, 2 batches):
    stage 1: Mt[j,i] = chebT[k][j,i]*attT[b][j,i] on DVE; PE matmuls
             R_T[tf, i] += xh[j, tf].T @ Mt[j, i] accumulated over j in PSUM.
    stage 2: PE matmuls out[i, (t,o)] += R_T[tf_blk, i].T @ thetap[k]
             with thetap a block-diagonal padded Theta ([128, 4*64] per k),
             accumulated over k in PSUM; fused ReLU on copy-out.

Host pre-processing: transpose att/cheb (so no on-device transposes are
needed), pack x as [j, t*F+f], build block-diag thetap, cast to bf16
(fp32 accumulate on device).
"""

import numpy as np

B, T, N, F_IN, F_OUT, K = 16, 12, 1024, 32, 64, 3
M_CORES = 8
NB = B // M_CORES          # batches per core
P = 128                    # SBUF partitions
NJ = N // P                # 8 contraction chunks
TF = T * F_IN              # 384
NTFB = TF // P             # 3 tf blocks
TBLK = P // F_IN           # 4 t's per tf block
IS = 512                   # stage-1 moving width
NIS = N // IS              # 2 i strips
TO = TBLK * F_OUT          # 256 = stage-2 rhs width

_cache = {}


def _build():
    import concourse.bacc as bacc
    import concourse.mybir as mybir
    import concourse.tile as tile

    DT = mybir.dt.bfloat16
    DTF = mybir.dt.float32

    nc = bacc.Bacc("TRN2", target_bir_lowering=False, debug=False)
    att_d = nc.dram_tensor("att_t", [NB, N, N], DT, kind="ExternalInput")
    xh_d = nc.dram_tensor("xh", [NB, N, TF], DT, kind="ExternalInput")
    cheb_d = nc.dram_tensor("cheb_t", [K, N, N], DT, kind="ExternalInput")
    thp_d = nc.dram_tensor("thetap", [K, P, TO], DT, kind="ExternalInput")
    out_d = nc.dram_tensor("out", [NB, T, N, F_OUT], DTF, kind="ExternalOutput")

    with tile.TileContext(nc) as tc:
        with (
            tc.tile_pool(name="cheb", bufs=1) as cheb_pool,
            tc.tile_pool(name="att", bufs=2) as att_pool,
            tc.tile_pool(name="xhp", bufs=2) as xh_pool,
            tc.tile_pool(name="mt", bufs=4) as mt_pool,
            tc.tile_pool(name="rt", bufs=2) as rt_pool,
            tc.tile_pool(name="thp", bufs=1) as thp_pool,
            tc.tile_pool(name="osb", bufs=3) as out_pool,
            tc.tile_pool(name="rtps", bufs=1, space="PSUM") as rtps_pool,
            tc.tile_pool(name="outps", bufs=1, space="PSUM") as outps_pool,
        ):
            # resident: chebT strips packed [128, NJ*N] per k, thetap
            cheb_sb = []
            for k in range(K):
                t_ = cheb_pool.tile([P, NJ * N], DT, tag=f"cheb{k}")
                nc.sync.dma_start(
                    t_[:].rearrange("p (jb i) -> p jb i", jb=NJ),
                    cheb_d.ap()[k].rearrange("(jb p) i -> p jb i", p=P),
                )
                cheb_sb.append(t_)
            thp_sb = thp_pool.tile([P, K * TO], DT, tag="thp")
            nc.sync.dma_start(
                thp_sb[:].rearrange("p (k n) -> p k n", k=K),
                thp_d.ap().rearrange("k p n -> p k n"),
            )

            for b in range(NB):
                att_sb = att_pool.tile([P, NJ * N], DT, tag="att")
                nc.sync.dma_start(
                    att_sb[:].rearrange("p (jb i) -> p jb i", jb=NJ),
                    att_d.ap()[b].rearrange("(jb p) i -> p jb i", p=P),
                )
                xh_sb = xh_pool.tile([P, NJ * TF], DT, tag="xh")
                nc.sync.dma_start(
                    xh_sb[:].rearrange("p (jb f) -> p jb f", jb=NJ),
                    xh_d.ap()[b].rearrange("(jb p) f -> p jb f", p=P),
                )

                # stage 1: R_T[k] [TF, N] for all k, bf16 in SBUF
                rt_sb = rt_pool.tile([P, K * NTFB * N], DT, tag="rt")
                for k in range(K):
                    rtps = [
                        rtps_pool.tile([P, IS], DTF, tag=f"rtps{q}", name=f"rtps{q}")
                        for q in range(NTFB * NIS)
                    ]
                    for j in range(NJ):
                        mt = mt_pool.tile([P, N], DT, tag="mt")
                        nc.vector.tensor_mul(
                            mt[:],
                            cheb_sb[k][:, j * N : (j + 1) * N],
                            att_sb[:, j * N : (j + 1) * N],
                        )
                        for tfb in range(NTFB):
                            lhs = xh_sb[:, j * TF + tfb * P : j * TF + (tfb + 1) * P]
                            for q in range(NIS):
                                nc.tensor.matmul(
                                    rtps[tfb * NIS + q][:],
                                    lhs,
                                    mt[:, q * IS : (q + 1) * IS],
                                    start=(j == 0),
                                    stop=(j == NJ - 1),
                                )
                    for tfb in range(NTFB):
                        for q in range(NIS):
                            base = (k * NTFB + tfb) * N + q * IS
                            nc.scalar.copy(
                                rt_sb[:, base : base + IS], rtps[tfb * NIS + q][:]
                            )

                # stage 2: out[i, (t,o)] accumulated over k per tf block
                for ic in range(NJ):
                    ops = outps_pool.tile([P, T * F_OUT], DTF, tag="outps")
                    for tfb in range(NTFB):
                        for k in range(K):
                            base = (k * NTFB + tfb) * N + ic * P
                            nc.tensor.matmul(
                                ops[:, tfb * TO : (tfb + 1) * TO],
                                rt_sb[:, base : base + P],
                                thp_sb[:, k * TO : (k + 1) * TO],
                                start=(k == 0),
                                stop=(k == K - 1),
                            )
                    osb = out_pool.tile([P, T * F_OUT], DTF, tag="osb")
                    nc.vector.tensor_relu(osb[:], ops[:])
                    nc.sync.dma_start(
                        out_d.ap()[b].rearrange("t (icc p) o -> icc p t o", p=P)[ic],
                        osb[:].rearrange("p (t o) -> p t o", t=T),
                    )

    nc.compile()
    return nc


def kernel(x, spatial_attention, cheb, Theta):
    from ml_dtypes import bfloat16
    from concourse.bass_utils import run_bass_kernel_spmd

    x = np.asarray(x, dtype=np.float32)
    att = np.asarray(spatial_attention, dtype=np.float32)
    cheb = np.asarray(cheb, dtype=np.float32)
    Theta = np.asarray(Theta, dtype=np.float32)

    if "nc" not in _cache:
        _cache["nc"] = _build()
    nc = _cache["nc"]

    attT = np.ascontiguousarray(att.transpose(0, 2, 1)).astype(bfloat16)
    chebT = np.ascontiguousarray(cheb.transpose(0, 2, 1)).astype(bfloat16)
    xh = np.ascontiguousarray(x.transpose(0, 2, 1, 3).reshape(B, N, TF)).astype(
        bfloat16
    )
    thetap = np.zeros((K, P, TO), dtype=np.float32)
    for tr in range(TBLK):
        thetap[:, tr * F_IN : (tr + 1) * F_IN, tr * F_OUT : (tr + 1) * F_OUT] = Theta
    thetap = thetap.astype(bfloat16)

    in_maps = [
        {
            "att_t": attT[c * NB : (c + 1) * NB],
            "xh": xh[c * NB : (c + 1) * NB],
            "cheb_t": chebT,
            "thetap": thetap,
        }
        for c in range(M_CORES)
    ]
    res = run_bass_kernel_spmd(nc, in_maps, list(range(M_CORES)))
    out = np.concatenate([res.results[c]["out"] for c in range(M_CORES)], axis=0)
    return out.astype(np.float32)
